# revision 1
# baseline (speedup 1.0000x reference)
"""CTC loss (mean, zero_infinity, target-length normalized) on 8 Trainium2 cores.

Sharding (per the hint): data-parallel over batch N — each core takes 8 of the
64 sequences and computes their per-sequence normalized NLL fully on device;
the host mean over the 64 values is the unshard step.

Device algorithm per core (exact log-space, numerically identical in
structure to the reference):
  Phase 1 (memory-bound): stream preds (T,8,C); ACT Exp+accumulate produces
    the per-(t,n) softmax denominator; a GPSIMD indirect_copy gathers the
    128 label columns per sequence; ACT Identity with per-partition bias
    forms lp(t,n,s) = pred_gathered - ln(sumexp); results stream to HBM in
    (t, n, s) layout (plus the blank column).
  Phase 2: per-t log-space CTC forward recursion on (8, L) state tiles:
    alpha'(s) = logaddexp3(alpha(s), alpha(s-1), alpha(s-2)+logskip(s)) + lp_t(s)
    with logaddexp3 = m + ln(e^(a-m)+e^(b-m)+e^(c-m)), m the 3-way max.
  Final: nll = -(logaddexp of the two end lanes) / len, on device.
"""
import sys
sys.path.insert(0, "/opt/trn_rl_repo")

import numpy as np

import concourse.bass as bass
import concourse.bacc as bacc
import concourse.tile as tile
from concourse import mybir
from concourse.bass_utils import run_bass_kernel_spmd

T_FULL, N_FULL, C, S = 1024, 64, 512, 128
L = 2 * S + 1
NCORES = 8
NL = N_FULL // NCORES
NEG = -1.0e9
F32 = mybir.dt.float32
U16 = mybir.dt.uint16
AF = mybir.ActivationFunctionType
OP = mybir.AluOpType
TBLK = 16            # DP t-block size for streaming lp slices

_COMPILED = {}


def build_program(T):
    nc = bacc.Bacc("TRN2", target_bir_lowering=False, debug=False)

    preds = nc.dram_tensor("preds", [T, NL, C], F32, kind="ExternalInput")
    gidx = nc.dram_tensor("gidx", [128, 64], U16, kind="ExternalInput")
    lskip = nc.dram_tensor("lskip", [NL, L], F32, kind="ExternalInput")
    e1m = nc.dram_tensor("e1m", [NL, L], F32, kind="ExternalInput")
    e2m = nc.dram_tensor("e2m", [NL, L], F32, kind="ExternalInput")
    invlen = nc.dram_tensor("invlen", [NL, 1], F32, kind="ExternalInput")
    nll = nc.dram_tensor("nll", [NL, 1], F32, kind="ExternalOutput")

    # lp lattice in HBM, (t, n, s) with s = 0 blank, 1..128 labels
    lpd = nc.dram_tensor("lpd", [T, NL, 132], F32)

    n_ttiles = T // 128

    with tile.TileContext(nc) as tc:
        with (
            tc.tile_pool(name="p1", bufs=2) as p1,
            tc.tile_pool(name="p1s", bufs=2) as p1s,
            tc.tile_pool(name="const", bufs=1) as constp,
            tc.tile_pool(name="dp", bufs=1) as dpp,
            tc.tile_pool(name="lps", bufs=3) as lpsp,
        ):
            # ---------------- constants ----------------
            t_gidx = constp.tile([128, 64], U16)
            nc.gpsimd.dma_start(t_gidx[:], gidx[:])
            t_lsk = constp.tile([NL, L], F32)
            nc.sync.dma_start(t_lsk[:], lskip[:])
            t_e1 = constp.tile([NL, L], F32)
            nc.sync.dma_start(t_e1[:], e1m[:])
            t_e2 = constp.tile([NL, L], F32)
            nc.sync.dma_start(t_e2[:], e2m[:])
            t_invl = constp.tile([NL, 1], F32)
            nc.sync.dma_start(t_invl[:], invlen[:])

            # ---------------- phase 1 ----------------
            for tt in range(n_ttiles):
                ts0 = tt * 128
                tp = p1.tile([128, NL * C], F32, tag="tp")
                nc.gpsimd.dma_start(
                    tp[:], preds[ts0:ts0 + 128].rearrange("t n c -> t (n c)"))
                es = p1s.tile([128, C], F32, tag="es")
                sm = p1s.tile([128, NL], F32, tag="sm")
                for n in range(NL):
                    nc.scalar.activation(es[:], tp[:, n * C:(n + 1) * C],
                                         AF.Exp, accum_out=sm[:, n:n + 1])
                lns = p1s.tile([128, NL], F32, tag="lns")
                nc.scalar.activation(lns[:], sm[:], AF.Ln)
                nb = p1s.tile([128, NL], F32, tag="nb")
                nc.vector.tensor_scalar(nb[:], lns[:], -1.0, 0.0,
                                        OP.mult, OP.add)
                g = p1s.tile([128, NL * S], F32, tag="g")
                nc.gpsimd.indirect_copy(g[:], tp[:], t_gidx[:], True)
                # lp tile (128, NL*132): [blank, 128 labels, pad, pad, pad]
                lp = p1s.tile([128, NL * 132], F32, tag="lp")
                lp3 = lp[:].rearrange("t (n k) -> t n k", k=132)
                nc.vector.memset(lp3[:, :, 129:132], 0.0)
                for n in range(NL):
                    nc.scalar.activation(
                        lp3[:, n, 1:129], g[:, n * S:(n + 1) * S],
                        AF.Identity, bias=nb[:, n:n + 1], scale=1.0)
                # blank column
                bl = p1s.tile([128, NL], F32, tag="bl")
                nc.vector.tensor_copy(
                    bl[:], tp[:].rearrange("t (n c) -> t n c", c=C)[:, :, 0])
                nc.vector.tensor_sub(bl[:], bl[:], lns[:])
                nc.vector.tensor_copy(lp3[:, :, 0], bl[:])
                nc.sync.dma_start(
                    lpd[ts0:ts0 + 128].rearrange("t n k -> t (n k)"), lp[:])

            # ---------------- phase 2: per-t log-space DP ----------------
            # state layout: cols 0,1 guard NEG; col 2+s = alpha(s)
            Wd = L + 2
            stA = dpp.tile([NL, Wd], F32)
            stB = dpp.tile([NL, Wd], F32)
            t1 = dpp.tile([NL, L], F32)
            t2 = dpp.tile([NL, L], F32)
            m = dpp.tile([NL, L], F32)
            e1 = dpp.tile([NL, L], F32)
            e2 = dpp.tile([NL, L], F32)
            e3 = dpp.tile([NL, L], F32)
            lpt = dpp.tile([NL, L], F32)
            nc.vector.memset(stA[:], NEG)
            nc.vector.memset(stB[:], NEG)
            state = [stA, stB]

            for blk in range(T // TBLK):
                lpb = lpsp.tile([NL, TBLK * 132], F32, tag="lpb")
                nc.sync.dma_start(
                    lpb[:].rearrange("n (t k) -> n t k", k=132),
                    lpd[blk * TBLK:(blk + 1) * TBLK].rearrange("t n k -> n t k"))
                for j in range(TBLK):
                    t = blk * TBLK + j
                    X = state[(t + 1) % 2]
                    Y = state[t % 2]
                    lps = lpb[:, j * 132:j * 132 + 132]
                    # build lp_t over lanes: even lanes = blank, odd = labels
                    # lpt[s even] = lps[0]... do via two strided copies
                    if t == 0:
                        # alpha(0, 0) = lp_blank(0); alpha(0, 1) = lp_label0(0)
                        nc.vector.tensor_copy(Y[:, 2:3], lps[:, 0:1])
                        nc.vector.tensor_copy(Y[:, 3:4], lps[:, 1:2])
                        continue
                    # m = max(alpha, alpha_sh1, alpha_sh2 + lskip)
                    nc.vector.tensor_max(t1[:], X[:, 2:2 + L], X[:, 1:1 + L])
                    nc.vector.tensor_add(t2[:], X[:, 0:L], t_lsk[:])
                    nc.vector.tensor_max(m[:], t1[:], t2[:])
                    # exps
                    nc.vector.tensor_sub(e1[:], X[:, 2:2 + L], m[:])
                    nc.scalar.activation(e1[:], e1[:], AF.Exp)
                    nc.vector.tensor_sub(e2[:], X[:, 1:1 + L], m[:])
                    nc.scalar.activation(e2[:], e2[:], AF.Exp)
                    nc.vector.tensor_sub(e3[:], t2[:], m[:])
                    nc.scalar.activation(e3[:], e3[:], AF.Exp)
                    nc.vector.tensor_add(e1[:], e1[:], e2[:])
                    nc.vector.tensor_add(e1[:], e1[:], e3[:])
                    nc.scalar.activation(e1[:], e1[:], AF.Ln)
                    nc.vector.tensor_add(m[:], m[:], e1[:])
                    # + lp_t : even lanes get blank col, odd lanes label cols
                    nc.vector.tensor_scalar_add(
                        Y[:, 2:2 + L:2], m[:, 0:L:2], lps[:, 0:1])
                    nc.vector.tensor_add(
                        Y[:, 3:2 + L:2], m[:, 1:L:2], lps[:, 1:129])
                X = state[(T - 1) % 2]

            # ---------------- final ----------------
            Xf = state[(T - 1) % 2]
            R1 = dpp.tile([NL, 1], F32)
            R2 = dpp.tile([NL, 1], F32)
            M1 = dpp.tile([NL, 1], F32)
            M2 = dpp.tile([NL, 1], F32)
            # masked extraction of the two end lanes (mask elsewhere ~0,
            # add NEG*(1-mask) is avoided by using max-reduce on alpha+mask)
            nc.vector.tensor_add(t1[:], Xf[:, 2:2 + L], t_e1[:])
            nc.vector.tensor_reduce(R1[:], t1[:], mybir.AxisListType.X, OP.max)
            nc.vector.tensor_add(t1[:], Xf[:, 2:2 + L], t_e2[:])
            nc.vector.tensor_reduce(R2[:], t1[:], mybir.AxisListType.X, OP.max)
            nc.vector.tensor_max(M1[:], R1[:], R2[:])
            nc.vector.tensor_tensor(M2[:], R1[:], R2[:], op=OP.min)
            nc.vector.tensor_sub(M2[:], M2[:], M1[:])
            nc.scalar.activation(M2[:], M2[:], AF.Exp)
            nc.vector.tensor_scalar_add(M2[:], M2[:], 1.0)
            nc.scalar.activation(M2[:], M2[:], AF.Ln)
            nc.vector.tensor_add(M1[:], M1[:], M2[:])
            nc.vector.tensor_scalar(M1[:], M1[:], -1.0, 0.0, OP.mult, OP.add)
            out = dpp.tile([NL, 1], F32)
            nc.vector.tensor_mul(out[:], M1[:], t_invl[:])
            nc.sync.dma_start(nll[:], out[:])

    nc.compile()
    return nc


def _host_prep(preds, labels, label_lengths, T):
    labels = np.asarray(labels).astype(np.int64)
    ll = np.asarray(label_lengths).astype(np.int64)
    in_maps = []
    for c in range(NCORES):
        ns = slice(c * NL, (c + 1) * NL)
        lab = labels[ns]
        lln = ll[ns]
        vals = np.zeros(1024, dtype=np.uint16)
        for n in range(NL):
            vals[n * S: (n + 1) * S] = (n * C + lab[n]).astype(np.uint16)
        wrap = np.zeros((16, 64), dtype=np.uint16)
        for j in range(1024):
            wrap[j % 16, j // 16] = vals[j]
        gidx = np.tile(wrap, (8, 1))
        lskip = np.full((NL, L), NEG, dtype=np.float32)
        for n in range(NL):
            for i in range(1, S):
                if lab[n, i] != lab[n, i - 1]:
                    lskip[n, 2 * i + 1] = 0.0
        # end-lane extraction masks: 0 at the end lane, NEG elsewhere
        e1 = np.full((NL, L), NEG, dtype=np.float32)
        e2 = np.full((NL, L), NEG, dtype=np.float32)
        for n in range(NL):
            e1[n, 2 * lln[n]] = 0.0
            e2[n, 2 * lln[n] - 1] = 0.0
        invlen = (1.0 / np.maximum(lln, 1)).astype(np.float32).reshape(NL, 1)
        in_maps.append({
            "preds": np.ascontiguousarray(preds[:, ns, :], dtype=np.float32),
            "gidx": gidx,
            "lskip": lskip,
            "e1m": e1,
            "e2m": e2,
            "invlen": invlen,
        })
    return in_maps


def run_device(preds, labels, label_lengths, T=T_FULL, trace=False):
    if T not in _COMPILED:
        _COMPILED[T] = build_program(T)
    nc = _COMPILED[T]
    in_maps = _host_prep(preds, labels, label_lengths, T)
    res = run_bass_kernel_spmd(nc, in_maps, list(range(NCORES)), trace=trace)
    nlls = np.concatenate([r["nll"].reshape(NL) for r in res.results])
    return nlls, res


def kernel(preds, labels, input_lengths, label_lengths):
    preds = np.asarray(preds)
    labels = np.asarray(labels)
    input_lengths = np.asarray(input_lengths)
    label_lengths = np.asarray(label_lengths)
    assert preds.shape == (T_FULL, N_FULL, C)
    assert int(input_lengths.min()) == T_FULL and int(input_lengths.max()) == T_FULL, \
        "kernel specialized for full-length inputs"
    nlls, _ = run_device(preds, labels, label_lengths)
    # zero_infinity: saturated/non-finite -> 0 (reference semantics)
    nlls = np.where(np.isfinite(nlls) & (np.abs(nlls) < 1e6), nlls, 0.0)
    return np.float32(np.mean(nlls))



# revision 30
# speedup vs baseline: 1.0424x; 1.0424x over previous
"""CTC loss (mean, zero_infinity, target-length normalized) on 8 Trainium2 cores.

Sharding: data-parallel over batch N — each core takes 8 of the 64 sequences,
computes per-sequence normalized NLL fully on device; host mean is the unshard.

Device algorithm per core (scaled exp-space forward algorithm):
  Phase 1 (memory-bound): stream preds (T,8,C); ACT Exp with accumulated
    per-(t,n) softmax denominator; GPSIMD indirect_copy gathers the 128 label
    columns from the exp'd tile; DVE normalizes to linear probabilities
    p(t,n,s) stored bf16 in HBM; blank probs go to a resident (16,T) f32 tile
    (rows 8-15 = 1.0) via an on-chip 32x32 transpose.
  Phase 2: per-t exp-space recursion on parity-split states stacked in 16
    partitions (rows 0-7 even lanes alpha(2j), rows 8-15 odd lanes alpha(2j+1)):
      E(j)  = Xe(j) + Xo(j-1)                      -> new even pre-prob
      O(j)  = Xo(j) + Xe(j) + m(j)*Xo(j-1)         -> new odd pre-prob
      Y     = (pre * pb_or_1) * P_t                (one fused stt op, rows
               0-7 scale by blank prob column, rows 8-15 by p_odd tensor)
    Renormalize every 8 steps by 1/sum(alpha) (sum from the fused accum_out);
    log-normalizers collect in a (8,127) buffer, summed once at the end:
      nll = -(ln(alpha_end_even + alpha_end_odd) + sum ln Z) / label_len
"""
import sys
sys.path.insert(0, "/opt/trn_rl_repo")

import numpy as np

import concourse.bass as bass
import concourse.bacc as bacc
import concourse.tile as tile
from concourse import mybir
from concourse.bass_utils import run_bass_kernel_spmd

T_FULL, N_FULL, C, S = 1024, 64, 512, 128
L = 2 * S + 1
NCORES = 8
NL = N_FULL // NCORES
NEG = -1.0e9
F32 = mybir.dt.float32
BF16 = mybir.dt.bfloat16
U16 = mybir.dt.uint16
AF = mybir.ActivationFunctionType
OP = mybir.AluOpType
TBLK = 32            # DP t-block size for streaming p slices
RK = 8               # renormalization period (steps)
OB = 32              # partition base of the odd-lane half (APs must start
                     # at partition 0/32/64/96)
W = 130              # per-t columns in the p lattice / state tiles
K0 = 845.0           # per-step prob pre-scale ~= e^6.74, the mean alpha
                     # decay/step; keeps the state near the renorm center so
                     # no f32/bf16 underflow between renorms. Removed at end.
LNCEN = 50.0         # renorm centers the state at e^+50 (not 1.0): buys 50
                     # extra nats of downward range for decaying feeder lanes
                     # whose late contributions otherwise flush to zero.
CEN = float(np.exp(LNCEN))

_COMPILED = {}


def build_program(T):
    nc = bacc.Bacc("TRN2", target_bir_lowering=False, debug=False)

    preds = nc.dram_tensor("preds", [T, NL, C], F32, kind="ExternalInput")
    gidx = nc.dram_tensor("gidx", [128, 64], U16, kind="ExternalInput")
    mpos = nc.dram_tensor("mpos", [NL, S], BF16, kind="ExternalInput")
    lmask = nc.dram_tensor("lmask", [128, NL * W], BF16, kind="ExternalInput")
    me = nc.dram_tensor("me", [NL, S + 1], BF16, kind="ExternalInput")
    mo = nc.dram_tensor("mo", [NL, S], BF16, kind="ExternalInput")
    minvl = nc.dram_tensor("minvl", [NL, 1], F32, kind="ExternalInput")
    nll = nc.dram_tensor("nll", [NL, 1], F32, kind="ExternalOutput")

    # linear-prob lattice in HBM: row n, time t, col 0 pad, cols 1..128 the
    # odd-lane (label) probs for states j=0..127, col 129 zero guard
    lpd = nc.dram_tensor("lpd", [NL, T, W], BF16)

    n_ttiles = T // 128
    nrenorm = (T - 1) // RK  # renorm steps: t = RK, 2*RK, ...
    renorms = set(range(RK, T, RK))

    with tile.TileContext(nc) as tc:
        with (
            tc.tile_pool(name="p1", bufs=2) as p1,
            tc.tile_pool(name="p1s", bufs=2) as p1s,
            tc.tile_pool(name="const", bufs=1) as constp,
            tc.tile_pool(name="dp", bufs=1) as dpp,
            tc.tile_pool(name="lps", bufs=3) as lpsp,
        ):
            # ---------------- constants ----------------
            t_gidx = constp.tile([128, 64], U16)
            nc.gpsimd.dma_start(t_gidx[:], gidx[:])
            t_mpos = constp.tile([NL, S], BF16)
            nc.sync.dma_start(t_mpos[:], mpos[:])
            t_lmask = constp.tile([128, NL * W], BF16)
            nc.sync.dma_start(t_lmask[:], lmask[:])
            t_me = constp.tile([NL, S + 1], BF16)
            nc.sync.dma_start(t_me[:], me[:])
            t_mo = constp.tile([NL, S], BF16)
            nc.sync.dma_start(t_mo[:], mo[:])
            t_invl = constp.tile([NL, 1], F32)
            nc.sync.dma_start(t_invl[:], minvl[:])

            # blank-prob lattice: rows 0..7 = pb(t,n) f32, resident
            pbs = constp.tile([NL, T], F32)

            # ---------------- phase 1 ----------------
            for tt in range(n_ttiles):
                ts0 = tt * 128
                tp = p1.tile([128, NL * C], F32, tag="tp")
                nc.gpsimd.dma_start(
                    tp[:], preds[:, :, :][ts0:ts0 + 128]
                    .rearrange("t n c -> t (n c)"))
                es = p1s.tile([128, NL * C], F32, tag="es")
                sm = p1s.tile([128, NL], F32, tag="sm")
                for n in range(NL):
                    nc.scalar.activation(es[:, n * C:(n + 1) * C],
                                         tp[:, n * C:(n + 1) * C],
                                         AF.Exp, accum_out=sm[:, n:n + 1])
                rs = p1s.tile([128, NL], F32, tag="rs")
                nc.vector.reciprocal(rs[:], sm[:])
                g = p1s.tile([128, NL * S], F32, tag="g")
                nc.gpsimd.indirect_copy(g[:], es[:], t_gidx[:], True)
                # normalized label probs, bf16, in (n, t, s) 130-col layout
                lp = p1s.tile([128, NL * W], BF16, tag="lp")
                nc.vector.memset(lp[:], 0.0)
                for n in range(NL):
                    nc.vector.tensor_scalar(
                        lp[:, n * W + 1:n * W + 1 + S],
                        g[:, n * S:(n + 1) * S],
                        rs[:, n:n + 1], K0, OP.mult, OP.mult)
                # zero lanes beyond each sequence's label length
                nc.vector.tensor_tensor(lp[:], lp[:], t_lmask[:], op=OP.mult)
                # blank probs: strided cols of es, normalized, then 32x32
                # transpose into the resident (8, T) pbs tile
                pt = p1s.tile([128, 32], F32, tag="pt")
                nc.vector.memset(pt[:], 0.0)
                es3 = es[:].rearrange("t (n c) -> t n c", c=C)
                nc.vector.scalar_tensor_tensor(pt[:, 0:NL], es3[:, :, 0], K0,
                                               rs[:], OP.mult, OP.mult)
                ptr = p1s.tile([128, 32], F32, tag="ptr")
                nc.vector.transpose(ptr[:], pt[:])
                for k in range(4):
                    nc.vector.tensor_copy(
                        pbs[0:NL, ts0 + 32 * k:ts0 + 32 * (k + 1)],
                        ptr[32 * k:32 * k + NL, :])
                nc.sync.dma_start(
                    lpd[:, ts0:ts0 + 128, :].rearrange("n t k -> t n k"),
                    lp[:].rearrange("t (n k) -> t n k", k=W))

            # ---------------- phase 2: exp-space DP ----------------
            # side-by-side parity state, one (8, 260) tile:
            #   cols 0..129   even lanes: col 0 guard, state j at col j+1
            #   cols 130..259 odd lanes:  col 130 guard, state j at col W+j+1
            stA = dpp.tile([NL, 2 * W], BF16)
            stB = dpp.tile([NL, 2 * W], BF16)
            tmp = dpp.tile([NL, 2 * W], BF16)
            t1 = dpp.tile([NL, S], BF16)
            zea = dpp.tile([NL, 1], F32)
            zob = dpp.tile([NL, 1], F32)
            zbuf = dpp.tile([NL, max(nrenorm, 1)], F32)
            rz = dpp.tile([NL, 1], F32)
            nc.vector.memset(stA[:], 0.0)
            nc.vector.memset(stB[:], 0.0)
            nc.vector.memset(tmp[:], 0.0)
            nc.vector.memset(zbuf[:], 1.0)
            state = [stA, stB]

            blk_tile = lpsp.tile([NL, TBLK * W], BF16, tag="pb")
            nc.sync.dma_start(
                blk_tile[:],
                lpd[:, 0:TBLK, :].rearrange("n t k -> n (t k)"))

            # init at t=0: alpha(s=0)=pb(0), alpha(s=1)=p_label0(0)
            nc.vector.tensor_copy(stA[:, 1:2], pbs[:, 0:1])
            nc.vector.tensor_copy(stA[:, W + 1:W + 2], blk_tile[:, 1:2])

            for t in range(1, T):
                if t % TBLK == 0:
                    blk_tile = lpsp.tile([NL, TBLK * W], BF16, tag="pb")
                    nc.sync.dma_start(
                        blk_tile[:],
                        lpd[:, t - t % TBLK:t - t % TBLK + TBLK, :]
                        .rearrange("n t k -> n (t k)"))
                j = t % TBLK
                X = state[(t + 1) % 2]
                Y = state[t % 2]
                rt = t in renorms
                if rt:
                    k = t // RK - 1
                    nc.vector.tensor_tensor(zbuf[:, k:k + 1], zea[:], zob[:],
                                            op=OP.add)
                    nc.vector.reciprocal(rz[:], zbuf[:, k:k + 1])
                    # fold the e^+50 recentering into the scale factor
                    nc.vector.tensor_scalar(rz[:], rz[:], CEN, None, OP.mult)
                # E(j) = Xe(j) + Xo(j-1)        [cols 1..129]
                nc.vector.scalar_tensor_tensor(
                    tmp[:, 1:W], X[:, 1:W], 1.0, X[:, W:2 * W - 1],
                    OP.mult, OP.add)
                # O(j) = Xo(j) + Xe(j)          [cols W+1..W+128]
                nc.vector.scalar_tensor_tensor(
                    tmp[:, W + 1:W + 1 + S], X[:, W + 1:W + 1 + S], 1.0,
                    X[:, 1:1 + S], OP.mult, OP.add)
                # t1 = m (.) Xo(j-1); O += t1
                nc.vector.tensor_tensor(t1[:], t_mpos[:], X[:, W:W + S],
                                        op=OP.mult)
                nc.vector.scalar_tensor_tensor(
                    tmp[:, W + 1:W + 1 + S], tmp[:, W + 1:W + 1 + S], 1.0,
                    t1[:], OP.mult, OP.add)
                # Ye = (E * pb) [* rz*CEN at renorm steps]
                acc = (t + 1) in renorms
                if rt:
                    nc.vector.tensor_scalar(
                        Y[:, 1:W], tmp[:, 1:W], pbs[:, t:t + 1],
                        rz[:, 0:1], OP.mult, OP.mult)
                else:
                    nc.vector.tensor_scalar(
                        Y[:, 1:W], tmp[:, 1:W], pbs[:, t:t + 1],
                        None, OP.mult)
                if acc:
                    nc.vector.tensor_reduce(zea[:], Y[:, 1:W],
                                            mybir.AxisListType.X, OP.add)
                # Yo = (O [* rz*CEN]) * p_odd
                nc.vector.scalar_tensor_tensor(
                    Y[:, W + 1:W + 1 + S], tmp[:, W + 1:W + 1 + S],
                    rz[:, 0:1] if rt else 1.0,
                    blk_tile[:, j * W + 1:j * W + 1 + S],
                    OP.mult, OP.mult,
                    accum_out=zob[:] if acc else None)

            # ---------------- final ----------------
            Xf = state[(T - 1) % 2]
            u = dpp.tile([NL, S + 1], BF16)
            v = dpp.tile([NL, S], BF16)
            re = dpp.tile([NL, 1], F32)
            ro = dpp.tile([NL, 1], F32)
            # scale by 2^-64 (exact) so Ln inputs are within the ACT range
            SC = float(2.0 ** -64)
            nc.vector.scalar_tensor_tensor(u[:], Xf[:, 1:W], SC, t_me[:],
                                           OP.mult, OP.mult)
            nc.vector.tensor_reduce(re[:], u[:], mybir.AxisListType.X, OP.add)
            nc.vector.scalar_tensor_tensor(v[:], Xf[:, W + 1:W + 1 + S], SC,
                                           t_mo[:], OP.mult, OP.mult)
            nc.vector.tensor_reduce(ro[:], v[:], mybir.AxisListType.X, OP.add)
            pt2 = dpp.tile([NL, 1], F32)
            nc.vector.tensor_tensor(pt2[:], re[:], ro[:], op=OP.add)
            lnp = dpp.tile([NL, 1], F32)
            nc.scalar.activation(lnp[:], pt2[:], AF.Ln)
            zs = dpp.tile([NL, max(nrenorm, 1)], F32)
            nc.vector.tensor_scalar(zs[:], zbuf[:], SC, None, OP.mult)
            lnz = dpp.tile([NL, max(nrenorm, 1)], F32)
            nc.scalar.activation(lnz[:], zs[:], AF.Ln)
            sz = dpp.tile([NL, 1], F32)
            nc.vector.tensor_reduce(sz[:], lnz[:], mybir.AxisListType.X,
                                    OP.add)
            tot = dpp.tile([NL, 1], F32)
            nc.vector.tensor_tensor(tot[:], lnp[:], sz[:], op=OP.add)
            out = dpp.tile([NL, 1], F32)
            cadj = float(64.0 * np.log(2.0) * (1 + len(renorms))
                         - T * np.log(K0) - len(renorms) * LNCEN)
            nc.vector.tensor_scalar(out[:], tot[:], cadj,
                                    t_invl[:, 0:1], OP.add, OP.mult)
            nc.sync.dma_start(nll[:], out[:])

    nc.compile()
    return nc


def _host_prep(preds, labels, label_lengths, T):
    labels = np.asarray(labels).astype(np.int64)
    ll = np.asarray(label_lengths).astype(np.int64)
    bf16 = mybir.dt.np(BF16)
    in_maps = []
    for c in range(NCORES):
        ns = slice(c * NL, (c + 1) * NL)
        lab = labels[ns]
        lln = ll[ns]
        vals = np.zeros(1024, dtype=np.uint16)
        for n in range(NL):
            vals[n * S: (n + 1) * S] = (n * C + lab[n]).astype(np.uint16)
        wrap = np.zeros((16, 64), dtype=np.uint16)
        for j in range(1024):
            wrap[j % 16, j // 16] = vals[j]
        gidx = np.tile(wrap, (8, 1))
        # skip-transition mask on odd lanes: m(j)=1 iff j>=1 and distinct label
        mpos = np.zeros((NL, S), dtype=np.float32)
        mpos[:, 1:] = (lab[:, 1:] != lab[:, :-1]).astype(np.float32)
        # lane mask: keep label lane j only while j < label_len
        lmrow = np.zeros((NL, W), dtype=np.float32)
        for n in range(NL):
            lmrow[n, 1:1 + lln[n]] = 1.0
        lmask = np.broadcast_to(lmrow.reshape(1, NL * W), (128, NL * W))
        # end-lane 0/1 masks
        me = np.zeros((NL, S + 1), dtype=np.float32)
        mo = np.zeros((NL, S), dtype=np.float32)
        for n in range(NL):
            me[n, lln[n]] = 1.0
            mo[n, lln[n] - 1] = 1.0
        minvl = (-1.0 / np.maximum(lln, 1)).astype(np.float32).reshape(NL, 1)
        in_maps.append({
            "preds": np.ascontiguousarray(preds[:, ns, :], dtype=np.float32),
            "gidx": gidx,
            "mpos": mpos.astype(bf16),
            "lmask": np.ascontiguousarray(lmask).astype(bf16),
            "me": me.astype(bf16),
            "mo": mo.astype(bf16),
            "minvl": minvl,
        })
    return in_maps


def run_device(preds, labels, label_lengths, T=T_FULL, trace=False):
    if T not in _COMPILED:
        _COMPILED[T] = build_program(T)
    nc = _COMPILED[T]
    in_maps = _host_prep(preds, labels, label_lengths, T)
    res = run_bass_kernel_spmd(nc, in_maps, list(range(NCORES)), trace=trace)
    nlls = np.concatenate([r["nll"].reshape(NL) for r in res.results])
    return nlls, res


def kernel(preds, labels, input_lengths, label_lengths):
    preds = np.asarray(preds)
    labels = np.asarray(labels)
    input_lengths = np.asarray(input_lengths)
    label_lengths = np.asarray(label_lengths)
    assert preds.shape == (T_FULL, N_FULL, C)
    assert int(input_lengths.min()) == T_FULL and int(input_lengths.max()) == T_FULL, \
        "kernel specialized for full-length inputs"
    nlls, _ = run_device(preds, labels, label_lengths)
    # zero_infinity: saturated/non-finite -> 0 (reference semantics)
    nlls = np.where(np.isfinite(nlls) & (np.abs(nlls) < 1e6), nlls, 0.0)
    return np.float32(np.mean(nlls))


# revision 46
# speedup vs baseline: 178.4695x; 171.2108x over previous
"""CTC loss (mean, zero_infinity, target-length normalized) on 8 Trainium2 cores.

Sharding: pairwise forward/backward split. Core pair (2p, 2p+1) owns 16
sequences. The even core runs the scaled exp-space FORWARD recursion over
t in [0, T/2); the odd core runs the BACKWARD recursion over t in [T/2, T),
expressed as the *same* forward-form program on host-reversed inputs
(preds reversed in t, labels/masks reversed in s; CTC's backward recursion in
reversed coordinates is exactly the forward recursion, with the end-lane mask
as the data-driven initial vector). Each core executes only T/2-1 sequential
DP steps. The pair exchanges final states + log-normalizer sums (~8 KB) via a
pairwise AllGather and combines with the exact split identity

    P = sum_s PRE_self(s) * reverse(state_partner)(s)
    nll = -(ln P + lnZ_fwd + lnZ_bwd - consts) / label_len

Numerics (all exact, not approximations):
  * exp-space scaled forward algorithm, bf16 state, renormalized to e^+50
    every 8 steps (50 extra nats of downward range for decaying lanes).
  * probabilities pre-scaled by K0 ~ e^6.74 (the mean per-step alpha decay)
    so the state is stationary between renorms.
  * label lanes beyond each sequence's label length are zeroed (they would
    otherwise dominate the renorm sum and flush the end lanes to zero).
  * every s-advance is weighted by rho = ll/(T/2 - ll) per sequence: a
    similarity transform (every fwd x bwd path pair carries exactly
    rho^(2*ll), removed by a per-sequence constant) that recenters both
    directions' probability bulks onto the meeting region at t = T/2 --
    without it the two supports are ~100+ nats apart in bf16 and the dot
    underflows to zero.

Device program per core:
  Phase 1 (memory-bound): stream the preds half (T/2, 16, C); ACT Exp with
    accumulated softmax denominators; GPSIMD indirect_copy gathers label
    columns from the exp'd tile; ACT rescales into a 260-col-per-t bf16
    lattice in HBM (cols 1..129 blank-prob broadcast, 131..258 label probs).
  Phase 2 (DVE-bound): per-t recursion on a side-by-side parity state
    (16, 260): E(j) = Xe(j) + rho*Xo(j-1); O(j) = Xo(j) + rho*Xe(j) +
    rho^2*m(j)*Xo(j-1); Y = pre * P_t (one tensor_tensor over 258 lanes).
    ~5 DVE ops per step; renorm reciprocal fused into the step scalars and
    the renorm sums taken by an ACT Copy accumulator.
"""
import sys
sys.path.insert(0, "/opt/trn_rl_repo")

import numpy as np

import concourse.bass as bass
import concourse.bacc as bacc
import concourse.tile as tile
from concourse import mybir
from concourse.bass_utils import run_bass_kernel_spmd

T_FULL, N_FULL, C, S = 1024, 64, 512, 128
L = 2 * S + 1
NCORES = 8
NPAIR = 16           # sequences per core pair (= per core, fwd+bwd split)
NH = 8               # half of NPAIR; phase 1 works in 8-sequence chunks
NEG = -1.0e9
F32 = mybir.dt.float32
BF16 = mybir.dt.bfloat16
U16 = mybir.dt.uint16
AF = mybir.ActivationFunctionType
OP = mybir.AluOpType
TBLK = 32            # DP t-block size for streaming p slices
RK = 8               # renormalization period (steps)
W = 130              # half-width of the per-t lattice layout (260 cols)
K0 = 845.0           # per-step prob pre-scale ~= e^6.74 (mean alpha decay)
LNCEN = 50.0         # renorm recenters the state at e^+50 for tail headroom
CEN = float(np.exp(LNCEN))
SC = float(2.0 ** -64)
LN2_64 = float(64.0 * np.log(2.0))

_COMPILED = {}


def build_program(TH):
    """One core's program: TH timesteps of lattice + TH-1 DP steps."""
    nc = bacc.Bacc("TRN2", target_bir_lowering=False, debug=False)

    preds = nc.dram_tensor("preds", [TH, NPAIR, C], F32, kind="ExternalInput")
    gidxa = nc.dram_tensor("gidxa", [128, 64], U16, kind="ExternalInput")
    gidxb = nc.dram_tensor("gidxb", [128, 64], U16, kind="ExternalInput")
    mpos = nc.dram_tensor("mpos", [NPAIR, S], BF16, kind="ExternalInput")
    lmask = nc.dram_tensor("lmask", [128, NPAIR * 2 * W], BF16,
                           kind="ExternalInput")
    ivec = nc.dram_tensor("ivec", [NPAIR, 2 * W], BF16, kind="ExternalInput")
    selm = nc.dram_tensor("selm", [NPAIR, 2], F32, kind="ExternalInput")
    rho = nc.dram_tensor("rho", [NPAIR, 1], F32, kind="ExternalInput")
    cads = nc.dram_tensor("cads", [NPAIR, 1], F32, kind="ExternalInput")
    minvl = nc.dram_tensor("minvl", [NPAIR, 1], F32, kind="ExternalInput")
    nll = nc.dram_tensor("nll", [NPAIR, 1], F32, kind="ExternalOutput")

    # linear-prob lattice in HBM: row n, time t, 260 cols:
    #   col 0 pad, 1..129 blank-prob broadcast, 130 zero guard,
    #   cols 131..258 label-lane probs j=0..127, col 259 zero
    lpd = nc.dram_tensor("lpd", [NPAIR, TH, 2 * W], BF16)
    # pair exchange staging: state (260 cols, f32) + lnz sum (col 260)
    xstage = nc.dram_tensor("xstage", [NPAIR, 261], F32)
    xgath = nc.dram_tensor("xgath", [2 * NPAIR, 261], F32)

    PTS = min(128, TH)       # phase-1 t-tile height
    n_ttiles = TH // PTS
    nrenorm = (TH - 1) // RK
    renorms = set(range(RK, TH, RK))

    with tile.TileContext(nc) as tc:
        with (
            tc.tile_pool(name="p1", bufs=2) as p1,
            tc.tile_pool(name="p1s", bufs=2) as p1s,
            tc.tile_pool(name="const", bufs=1) as constp,
            tc.tile_pool(name="dp", bufs=1) as dpp,
            tc.tile_pool(name="lps", bufs=3) as lpsp,
        ):
            # ---------------- constants ----------------
            t_gidxa = constp.tile([128, 64], U16, tag="gidxa")
            t_gidxb = constp.tile([128, 64], U16, tag="gidxb")
            t_gidx = [t_gidxa, t_gidxb]
            nc.gpsimd.dma_start(t_gidx[0][:], gidxa[:])
            nc.gpsimd.dma_start(t_gidx[1][:], gidxb[:])
            t_mpos = constp.tile([NPAIR, S], BF16)
            nc.sync.dma_start(t_mpos[:], mpos[:])
            t_lmask = constp.tile([128, NPAIR * 2 * W], BF16)
            nc.sync.dma_start(t_lmask[:], lmask[:])
            t_ivec = constp.tile([NPAIR, 2 * W], BF16)
            nc.sync.dma_start(t_ivec[:], ivec[:])
            t_selm = constp.tile([NPAIR, 2], F32)
            nc.sync.dma_start(t_selm[:], selm[:])
            t_rho = constp.tile([NPAIR, 1], F32)
            nc.sync.dma_start(t_rho[:], rho[:])
            t_cads = constp.tile([NPAIR, 1], F32)
            nc.sync.dma_start(t_cads[:], cads[:])
            t_invl = constp.tile([NPAIR, 1], F32)
            nc.sync.dma_start(t_invl[:], minvl[:])
            ones = constp.tile([128, W - 1], BF16)
            nc.vector.memset(ones[:], 1.0)

            # ---------------- phase 1 ----------------
            # (tt, h): 128 timesteps x 8 sequences at a time
            for tt in range(n_ttiles):
                ts0 = tt * PTS
                lp = p1s.tile([PTS, NPAIR * 2 * W], BF16, tag="lp")
                nc.vector.memset(lp[:], 0.0)
                for h in range(2):
                    n0 = h * NH
                    tp = p1.tile([PTS, NH * C], F32, tag="tp")
                    nc.gpsimd.dma_start(
                        tp[:], preds[:, :, :][ts0:ts0 + PTS, n0:n0 + NH]
                        .rearrange("t n c -> t (n c)"))
                    es = p1s.tile([PTS, NH * C], F32, tag="es")
                    sm = p1s.tile([PTS, NH], F32, tag="sm")
                    for n in range(NH):
                        nc.scalar.activation(es[:, n * C:(n + 1) * C],
                                             tp[:, n * C:(n + 1) * C],
                                             AF.Exp, accum_out=sm[:, n:n + 1])
                    rs = p1s.tile([PTS, NH], F32, tag="rs")
                    nc.vector.reciprocal(rs[:], sm[:])
                    rsk = p1s.tile([PTS, NH], F32, tag="rsk")
                    nc.vector.tensor_scalar(rsk[:], rs[:], K0, None, OP.mult)
                    g = p1s.tile([PTS, NH * S], F32, tag="g")
                    nc.gpsimd.indirect_copy(g[:], es[:], t_gidx[h][0:PTS],
                                            True)
                    # per-(t,n) blank probs (K0-scaled)
                    pt = p1s.tile([PTS, NH], F32, tag="pt")
                    es3 = es[:].rearrange("t (n c) -> t n c", c=C)
                    nc.vector.scalar_tensor_tensor(pt[:], es3[:, :, 0], K0,
                                                   rs[:], OP.mult, OP.mult)
                    for n in range(NH):
                        c0 = (n0 + n) * 2 * W
                        nc.scalar.activation(
                            lp[:, c0 + 1:c0 + W], ones[0:PTS, 0:W - 1],
                            AF.Copy, scale=pt[:, n:n + 1])
                        nc.scalar.activation(
                            lp[:, c0 + W + 1:c0 + W + 1 + S],
                            g[:, n * S:(n + 1) * S],
                            AF.Copy, scale=rsk[:, n:n + 1])
                # zero label lanes beyond each sequence's label length
                nc.vector.tensor_tensor(lp[:], lp[:], t_lmask[0:PTS, :],
                                        op=OP.mult)
                nc.sync.dma_start(
                    lpd[:, ts0:ts0 + PTS, :].rearrange("n t k -> t n k"),
                    lp[:].rearrange("t (n k) -> t n k", k=2 * W))

            # ---------------- phase 2: exp-space DP ----------------
            stA = dpp.tile([NPAIR, 2 * W], BF16)
            stB = dpp.tile([NPAIR, 2 * W], BF16)
            tmp = dpp.tile([NPAIR, 2 * W], BF16)
            t1 = dpp.tile([NPAIR, S], BF16)
            zbuf = dpp.tile([NPAIR, max(nrenorm, 1)], F32)
            rz = dpp.tile([NPAIR, 1], F32)
            zscr = dpp.tile([NPAIR, 2 * W - 2], BF16)
            nc.vector.memset(stA[:], 0.0)
            nc.vector.memset(stB[:], 0.0)
            nc.vector.memset(tmp[:], 0.0)
            nc.vector.memset(zbuf[:], 1.0)
            state = [stA, stB]

            blk_tile = lpsp.tile([NPAIR, TBLK * 2 * W], BF16, tag="pb")
            nc.sync.dma_start(
                blk_tile[:],
                lpd[:, 0:TBLK, :].rearrange("n t k -> n (t k)"))

            # data-driven init: state0 = P_0 (.) ivec
            # (fwd: delta at lanes s=0, s=1; bwd: the end-lane mask)
            nc.vector.tensor_tensor(stA[:, 1:2 * W - 1],
                                    blk_tile[:, 1:2 * W - 1],
                                    t_ivec[:, 1:2 * W - 1], op=OP.mult)

            for t in range(1, TH):
                if t % TBLK == 0:
                    blk_tile = lpsp.tile([NPAIR, TBLK * 2 * W], BF16,
                                         tag="pb")
                    nc.sync.dma_start(
                        blk_tile[:],
                        lpd[:, t - t % TBLK:t - t % TBLK + TBLK, :]
                        .rearrange("n t k -> n (t k)"))
                j = t % TBLK
                X = state[(t + 1) % 2]
                Y = state[t % 2]
                rt = t in renorms
                if rt:
                    k = t // RK - 1
                    nc.vector.reciprocal(rz[:], zbuf[:, k:k + 1])
                    nc.vector.tensor_scalar(rz[:], rz[:], CEN, None, OP.mult)
                # E(j) = Xe(j) + rho*Xo(j-1)       [cols 1..129]
                nc.vector.scalar_tensor_tensor(
                    tmp[:, 1:W], X[:, W:2 * W - 1], t_rho[:, 0:1],
                    X[:, 1:W], OP.mult, OP.add)
                # O(j) = Xo(j) + rho*Xe(j)         [cols W+1..W+128]
                nc.vector.scalar_tensor_tensor(
                    tmp[:, W + 1:W + 1 + S], X[:, 1:1 + S], t_rho[:, 0:1],
                    X[:, W + 1:W + 1 + S], OP.mult, OP.add)
                # t1 = m (.) Xo(j-1); O += t1
                nc.vector.tensor_tensor(t1[:], t_mpos[:], X[:, W:W + S],
                                        op=OP.mult)
                nc.vector.tensor_tensor(
                    tmp[:, W + 1:W + 1 + S], tmp[:, W + 1:W + 1 + S],
                    t1[:], op=OP.add)
                if rt:
                    nc.vector.tensor_scalar(
                        tmp[:, 1:2 * W - 1], tmp[:, 1:2 * W - 1],
                        rz[:, 0:1], None, OP.mult)
                # Y = pre * P_t over all 258 lanes
                nc.vector.tensor_tensor(
                    Y[:, 1:2 * W - 1], tmp[:, 1:2 * W - 1],
                    blk_tile[:, j * 2 * W + 1:(j + 1) * 2 * W - 1],
                    op=OP.mult)
                if (t + 1) in renorms:
                    k2 = (t + 1) // RK - 1
                    nc.scalar.activation(zscr[:], Y[:, 1:2 * W - 1],
                                         AF.Copy,
                                         accum_out=zbuf[:, k2:k2 + 1])

            # ---------------- half-step PRE + pair exchange ----------------
            Xf = state[(TH - 1) % 2]
            # TMP = weighted PRE for the (virtual) next step
            nc.vector.scalar_tensor_tensor(
                tmp[:, 1:W], Xf[:, W:2 * W - 1], t_rho[:, 0:1],
                Xf[:, 1:W], OP.mult, OP.add)
            nc.vector.scalar_tensor_tensor(
                tmp[:, W + 1:W + 1 + S], Xf[:, 1:1 + S], t_rho[:, 0:1],
                Xf[:, W + 1:W + 1 + S], OP.mult, OP.add)
            nc.vector.tensor_tensor(t1[:], t_mpos[:], Xf[:, W:W + S],
                                    op=OP.mult)
            nc.vector.tensor_tensor(
                tmp[:, W + 1:W + 1 + S], tmp[:, W + 1:W + 1 + S],
                t1[:], op=OP.add)
            tmps = dpp.tile([NPAIR, 2 * W], F32)
            nc.vector.memset(tmps[:], 0.0)
            nc.vector.tensor_scalar(tmps[:, 1:2 * W - 1],
                                    tmp[:, 1:2 * W - 1], SC, None, OP.mult)
            # own lnz sum: Ln(z * 2^-64) summed
            zs = dpp.tile([NPAIR, max(nrenorm, 1)], F32)
            nc.vector.tensor_scalar(zs[:], zbuf[:], SC, None, OP.mult)
            lnz = dpp.tile([NPAIR, max(nrenorm, 1)], F32)
            nc.scalar.activation(lnz[:], zs[:], AF.Ln)
            # staging tile: cols 0..259 = state REVERSED within each parity
            # half (so the partner's dot needs no reversed reads), col 260 =
            # lnz sum
            stg = dpp.tile([NPAIR, 261], F32)
            nc.vector.memset(stg[:], 0.0)
            nc.vector.tensor_copy(stg[:, 1:W], Xf[:, W - 1:0:-1])
            nc.vector.tensor_copy(stg[:, W + 1:W + 1 + S],
                                  Xf[:, 2 * W - 2:W:-1])
            nc.vector.tensor_reduce(stg[:, 260:261], lnz[:],
                                    mybir.AxisListType.X, OP.add)
            nc.sync.dma_start(xstage[:], stg[:])
            nc.gpsimd.collective_compute(
                "AllGather", OP.bypass,
                replica_groups=[[0, 1], [2, 3], [4, 5], [6, 7]],
                ins=[xstage[:]], outs=[xgath[:]])
            xg0 = dpp.tile([NPAIR, 261], F32)
            xg1 = dpp.tile([NPAIR, 261], F32)
            nc.sync.dma_start(xg0[:], xgath[0:NPAIR, :])
            nc.sync.dma_start(xg1[:], xgath[NPAIR:2 * NPAIR, :])

            # ---------------- combine ----------------
            # dot(TMPs, reverse(partner state)) per parity half; both
            # candidate partners computed, selected by the selm input.
            u0 = dpp.tile([NPAIR, 2 * W], F32)
            u1 = dpp.tile([NPAIR, 2 * W], F32)
            nc.vector.memset(u0[:], 0.0)
            nc.vector.memset(u1[:], 0.0)
            d0 = dpp.tile([NPAIR, 1], F32)
            d1 = dpp.tile([NPAIR, 1], F32)
            de = dpp.tile([NPAIR, 2], F32)
            do = dpp.tile([NPAIR, 2], F32)
            for i, (u, d, xg) in enumerate(((u0, d0, xg1), (u1, d1, xg0))):
                # partner state arrives pre-reversed: plain aligned products;
                # the dot sums come from the stt accumulators directly
                nc.vector.scalar_tensor_tensor(
                    u[:, 1:W], tmps[:, 1:W], SC,
                    xg[:, 1:W], OP.mult, OP.mult,
                    accum_out=de[:, i:i + 1])
                nc.vector.scalar_tensor_tensor(
                    u[:, W + 1:W + 1 + S], tmps[:, W + 1:W + 1 + S], SC,
                    xg[:, W + 1:W + 1 + S], OP.mult, OP.mult,
                    accum_out=do[:, i:i + 1])
                nc.vector.tensor_tensor(d[:], de[:, i:i + 1],
                                        do[:, i:i + 1], op=OP.add)
            # P = sel0*d0 + sel1*d1
            pt2 = dpp.tile([NPAIR, 1], F32)
            nc.vector.tensor_tensor(pt2[:], d0[:], t_selm[:, 0:1], op=OP.mult)
            nc.vector.scalar_tensor_tensor(
                pt2[:], d1[:], t_selm[:, 1:2], pt2[:], OP.mult, OP.add)
            lnp = dpp.tile([NPAIR, 1], F32)
            nc.scalar.activation(lnp[:], pt2[:], AF.Ln)
            # total = lnP + lnz_self + lnz_partner (slices cover both)
            tot = dpp.tile([NPAIR, 1], F32)
            nc.vector.tensor_tensor(tot[:], lnp[:], xg0[:, 260:261],
                                    op=OP.add)
            nc.vector.tensor_tensor(tot[:], tot[:], xg1[:, 260:261],
                                    op=OP.add)
            out = dpp.tile([NPAIR, 1], F32)
            cadj = float(LN2_64 * (2 + 2 * nrenorm)
                         - 2 * nrenorm * LNCEN - 2 * TH * np.log(K0))
            nc.vector.tensor_scalar(out[:], tot[:], cadj,
                                    t_invl[:, 0:1], OP.add, OP.mult)
            nc.vector.tensor_tensor(out[:], out[:], t_cads[:], op=OP.add)
            nc.sync.dma_start(nll[:], out[:])

    nc.compile()
    return nc


def _mk_gidx(lab8):
    """Gather table for 8 sequences: wrap-16 layout of n*C + label."""
    vals = np.zeros(1024, dtype=np.uint16)
    for n in range(NH):
        vals[n * S:(n + 1) * S] = (n * C + lab8[n]).astype(np.uint16)
    wrap = np.zeros((16, 64), dtype=np.uint16)
    for j in range(1024):
        wrap[j % 16, j // 16] = vals[j]
    return np.tile(wrap, (8, 1))


def _mk_lmask(keep):
    """260-col lane mask; keep[n] is a bool (S,) vector per sequence."""
    nn = keep.shape[0]
    lmrow = np.zeros((nn, 2 * W), dtype=np.float32)
    lmrow[:, 1:W] = 1.0
    lmrow[:, W + 1:W + 1 + S] = keep.astype(np.float32)
    return np.broadcast_to(lmrow.reshape(1, nn * 2 * W), (128, nn * 2 * W))


def _host_prep(preds, labels, label_lengths, T):
    TH = T // 2
    labels = np.asarray(labels).astype(np.int64)
    ll = np.asarray(label_lengths).astype(np.int64)
    bf16 = mybir.dt.np(BF16)
    jj = np.arange(S)
    in_maps = []
    for c in range(NCORES):
        p = c // 2
        bwd = c % 2 == 1
        ns = slice(p * NPAIR, (p + 1) * NPAIR)
        lab = labels[ns]
        lln = ll[ns]
        # advance weight: recenters the fwd/bwd bulks onto the meeting
        # region at t=TH; exact (every fwd*bwd path pair carries rho^2ll)
        rhov = (lln / np.maximum(TH - lln, 1)).astype(np.float32)
        rhov = np.clip(rhov, 1e-3, 1.0)
        if not bwd:
            pr = preds[0:TH, ns, :]
            labx = lab
            # m(j) = 1 iff j>=1 and distinct label; carries the rho^2 of
            # the double advance
            mpos = np.zeros((NPAIR, S), dtype=np.float32)
            mpos[:, 1:] = (lab[:, 1:] != lab[:, :-1]).astype(np.float32)
            mpos *= (rhov ** 2)[:, None]
            keep = jj[None, :] < lln[:, None]
            ivec = np.zeros((NPAIR, 2 * W), dtype=np.float32)
            ivec[:, 1] = 1.0
            for n in range(NPAIR):
                ivec[n, W + 1] = rhov[n]
            sel = np.tile(np.array([[1.0, 0.0]], np.float32), (NPAIR, 1))
        else:
            pr = preds[T - 1:TH - 1:-1, ns, :]
            labx = lab[:, ::-1]
            # reversed-coords skip mask: m'(j') = m(128-j')
            mpos = np.zeros((NPAIR, S), dtype=np.float32)
            jo = 128 - jj  # original odd lane index
            valid = (jo >= 1) & (jo <= S - 1)
            for n in range(NPAIR):
                mn = np.zeros(S, np.float32)
                mn[valid] = (lab[n, jo[valid]] != lab[n, jo[valid] - 1])
                mpos[n] = mn
            mpos *= (rhov ** 2)[:, None]
            # reversed junk lanes (j' <= 127-ll) are unreachable; keep all
            keep = np.ones((NPAIR, S), dtype=bool)
            ivec = np.zeros((NPAIR, 2 * W), dtype=np.float32)
            for n in range(NPAIR):
                jp = 128 - lln[n]
                ivec[n, 1 + jp] = 1.0              # even end lane
                ivec[n, W + 1 + jp] = rhov[n]      # odd end lane
            sel = np.tile(np.array([[0.0, 1.0]], np.float32), (NPAIR, 1))
        minvl = (-1.0 / np.maximum(lln, 1)).astype(np.float32).reshape(
            NPAIR, 1)
        in_maps.append({
            "preds": np.ascontiguousarray(pr, dtype=np.float32),
            "gidxa": _mk_gidx(labx[0:NH]),
            "gidxb": _mk_gidx(labx[NH:NPAIR]),
            "mpos": mpos.astype(bf16),
            "lmask": np.ascontiguousarray(_mk_lmask(keep)).astype(bf16),
            "ivec": ivec.astype(bf16),
            "selm": sel,
            "rho": rhov.reshape(NPAIR, 1),
            "cads": (2.0 * np.log(rhov)).astype(np.float32).reshape(NPAIR, 1),
            "minvl": minvl,
        })
    return in_maps


def run_device(preds, labels, label_lengths, T=T_FULL, trace=False):
    TH = T // 2
    if TH not in _COMPILED:
        _COMPILED[TH] = build_program(TH)
    nc = _COMPILED[TH]
    in_maps = _host_prep(preds, labels, label_lengths, T)
    res = run_bass_kernel_spmd(nc, in_maps, list(range(NCORES)), trace=trace)
    # even cores carry their pair's result (odd cores compute the same value)
    nlls = np.concatenate([res.results[2 * p]["nll"].reshape(NPAIR)
                           for p in range(NCORES // 2)])
    return nlls, res


def kernel(preds, labels, input_lengths, label_lengths):
    preds = np.asarray(preds)
    labels = np.asarray(labels)
    input_lengths = np.asarray(input_lengths)
    label_lengths = np.asarray(label_lengths)
    assert preds.shape == (T_FULL, N_FULL, C)
    assert int(input_lengths.min()) == T_FULL and int(input_lengths.max()) == T_FULL, \
        "kernel specialized for full-length inputs"
    nlls, _ = run_device(preds, labels, label_lengths)
    # zero_infinity: saturated/non-finite -> 0 (reference semantics)
    nlls = np.where(np.isfinite(nlls) & (np.abs(nlls) < 1e6), nlls, 0.0)
    return np.float32(np.mean(nlls))
